# revision 16
# baseline (speedup 1.0000x reference)
"""Trainium2 Bass kernel for nn_BiInteraction (segment softmax bi-interaction).

Strategy (data-parallel over molecules, 8 NeuronCores):
  - Each core owns 8 molecules. protSeq is DMA'd ONCE per core in protT
    layout (d on partitions), one DMA per 2-molecule stack spread over 4
    queues (sync/scalar/gpsimd/vector) so every stack lands ~simultaneously;
    the natural layout needed for residue pooling is derived ON-CHIP
    (saves ~1MB of HBM traffic per core and removes the late-landing DMA
    the pools used to wait on).
  - All on-chip transposes are REGULAR matmuls against the identity, not
    transpose-mode ops: transpose-mode does not count as PE-busy for the
    HAM clock gate, and a cold PE runs at half clock.
  - Atoms padded to 64 slots per molecule (pads replicate a real atom so
    max reductions stay exact); indicator columns handle segment sums.
  - All matmul operands fp16 (PSUM accumulation fp32).
  - Scores S[a, l] block-diagonal, 2 molecules stacked per PSUM bank so
    the two 512-col matmuls run concurrently on different col-groups.
  - Residue pools quadrant-packed via tile_position (4 concurrent MMs).
  - MLP in a single group of 8 molecules (half the LDWEIGHTS of a split).
  - PE warm-up + filler matmuls keep the HAM activity window busy from
    kernel start through the DMA-gated phase.

All shapes static and identical across cores (single SPMD program).
"""

import numpy as np

import concourse.bacc as bacc
import concourse.bass as bass
import concourse.tile as tile
from concourse import mybir
from concourse.bass_utils import run_bass_kernel_spmd

F32 = mybir.dt.float32
F16 = mybir.dt.float16
AxX = mybir.AxisListType.X
AF = mybir.ActivationFunctionType

A, L, D, B = 2048, 512, 128, 64
H1, H2 = 512, 256
NCORES = 8
MPC = B // NCORES            # molecules per core = 8
NPAD = 64                    # padded atom slots per molecule
NSTACK = MPC * NPAD // 128   # stacks of 128 padded atoms per core = 4
WARM_MM = 7                  # PE warm-up matmuls (256 cols each)

# fp16 consts column layout (after atomT | watt in the "early" tensor)
C_IDENT = 0        # [0, 128)   identity
C_IND = 128        # [128, 136) indicator, col = molecule
C_ONES = 136       # [136, 137) ones column
C_WO = 137         # [137, 139) Wo chunks
C_W = 139

AW_W = MPC * NPAD + D + C_W  # early tensor width = 779

_PROGRAM_CACHE = {}


def _build_program():
    nc = bacc.Bacc("TRN2", target_bir_lowering=False, debug=False)

    d_early = nc.dram_tensor("early", [128, AW_W], F16, kind="ExternalInput")
    d_protp = [
        nc.dram_tensor(f"protp{s}", [128, 2 * L], F16, kind="ExternalInput")
        for s in range(NSTACK)
    ]
    d_w12 = nc.dram_tensor("w12", [128, 2 * H1 + 4 * H2], F16, kind="ExternalInput")
    d_atomn = nc.dram_tensor("atomn", [128, NSTACK * D], F16, kind="ExternalInput")
    d_bias = nc.dram_tensor("biasc", [128, 8], F32, kind="ExternalInput")
    d_y = nc.dram_tensor("y", [MPC, 1], F32, kind="ExternalOutput")
    d_warm = nc.dram_tensor("warmo", [1, 1], F32, kind="ExternalOutput")

    with tile.TileContext(nc) as tc:
        with (
            tc.tile_pool(name="weights", bufs=1) as wpool,
            tc.tile_pool(name="work", bufs=1) as work,
            tc.tile_pool(name="spool", bufs=2) as spool,
            tc.tile_pool(name="psum_big", bufs=2, space=bass.MemorySpace.PSUM) as pbig,
            tc.tile_pool(name="psum_f16", bufs=2, space=bass.MemorySpace.PSUM) as pf,
            tc.tile_pool(name="psum_s", bufs=2, space=bass.MemorySpace.PSUM) as ps,
        ):
            # ---- loads: 3 queues, per-queue FIFO, earliest-needed first
            early = wpool.tile([128, AW_W], F16)
            nc.sync.dma_start(early[:], d_early[:])
            protp0 = wpool.tile([128, 2 * L], F16)
            nc.scalar.dma_start(protp0[:], d_protp[0][:])
            protp1 = wpool.tile([128, 2 * L], F16)
            nc.gpsimd.dma_start(protp1[:], d_protp[1][:])
            protp2 = wpool.tile([128, 2 * L], F16)
            nc.sync.dma_start(protp2[:], d_protp[2][:])
            biasc = wpool.tile([128, 8], F32)
            nc.scalar.dma_start(biasc[:], d_bias[:])
            w12 = wpool.tile([128, 2 * H1 + 4 * H2], F16)
            nc.scalar.dma_start(w12[:], d_w12[:])
            protp3 = wpool.tile([128, 2 * L], F16)
            nc.gpsimd.dma_start(protp3[:], d_protp[3][:])
            atomn = wpool.tile([128, NSTACK * D], F16)
            nc.gpsimd.dma_start(atomn[:], d_atomn[:])

            # vector: memsets (no DMA capability on DVE)
            warm = work.tile([128, 256], F16)
            nc.vector.memset(warm[:], 0.0)
            row1 = work.tile([1, 128], F16)
            nc.vector.memset(row1[:], 1.0)

            protps = [protp0, protp1, protp2, protp3]
            protT = [
                protps[i // 2][:, (i % 2) * L : (i % 2 + 1) * L] for i in range(MPC)
            ]
            atomT = early[:, 0 : MPC * NPAD]
            watt = early[:, MPC * NPAD : MPC * NPAD + D]
            consts = early[:, MPC * NPAD + D :]
            ident = consts[:, C_IDENT : C_IDENT + 128]
            ones_col = consts[:, C_ONES : C_ONES + 1]
            atomN = atomn[:].rearrange("p (s d) -> p s d", s=NSTACK)
            w1 = w12[:, 0 : 2 * H1]
            w2 = w12[:, 2 * H1 :]

            # ---- HAM warm-up: bridge kernel start -> first data --------
            ps_warm = ps.tile([128, 256], F32, tag="warm", bufs=1)
            for _ in range(WARM_MM):
                nc.tensor.matmul(
                    ps_warm[:], warm[:, :128], warm[:], start=True, stop=True
                )
            warm_out = work.tile([1, 1], F32)
            nc.vector.tensor_copy(warm_out[:], ps_warm[0:1, 0:1])
            nc.sync.dma_start(d_warm[:], warm_out[:])

            def filler(n=1):
                # keep the HAM activity window busy across DMA-gated gaps
                for _ in range(n):
                    nc.tensor.matmul(
                        ps_warm[:, 0:128], warm[:, :128], warm[:, :128],
                        start=True, stop=True,
                    )

            # ---- XT = W_att.T-applied atoms: XT[d', a] -----------------
            ps_xt = pf.tile([128, MPC * NPAD], F32, tag="pn")
            nc.tensor.matmul(ps_xt[:], watt[:], atomT[:], start=True, stop=True)
            xt = work.tile([128, MPC * NPAD], F16)
            nc.vector.tensor_copy(xt[:, 0:256], ps_xt[:, 0:256])
            nc.scalar.copy(xt[:, 256:512], ps_xt[:, 256:512])
            filler(2)

            # ---- per-stack: pnat, scores, S^T, maxes, exp --------------
            # wpe col layout per stack s: col 9s = Wc; cols 9s+1+2j+sl = Wp
            pnat = wpool.tile([128, MPC * L], F16)   # natural-layout prot
            wpe = work.tile([128, 9 * NSTACK], F32)
            ewx = work.tile([128, 9 * NSTACK], F16)
            wce = work.tile([128, NSTACK], F32)
            wcseg = work.tile([128, MPC], F16)
            for s in range(NSTACK):
                # natural-layout prot for this stack's molecules (regular
                # matmul vs identity: PE-transpose that feeds HAM)
                for slot in range(2):
                    i = 2 * s + slot
                    ps_pn = pf.tile([128, L], F32, tag="pn")
                    for j in range(4):
                        nc.tensor.matmul(
                            ps_pn[:, j * 128 : (j + 1) * 128],
                            protT[i][:, j * 128 : (j + 1) * 128],
                            ident,
                            start=True,
                            stop=True,
                        )
                    nc.vector.tensor_copy(
                        pnat[:, i * L : i * L + 256], ps_pn[:, 0:256]
                    )
                    nc.scalar.copy(
                        pnat[:, i * L + 256 : i * L + L], ps_pn[:, 256:512]
                    )
                ps_S = pbig.tile([128, L], F32, tag="big")
                for slot in range(2):
                    i = 2 * s + slot
                    nc.tensor.matmul(
                        ps_S[slot * NPAD : (slot + 1) * NPAD, :],
                        xt[:, i * NPAD : (i + 1) * NPAD],
                        protT[i],
                        start=True,
                        stop=True,
                    )
                s_sb = spool.tile([128, L], F16, tag="s_sb")
                nc.scalar.copy(s_sb[:, 0:256], ps_S[:, 0:256])
                nc.scalar.copy(s_sb[:, 256:512], ps_S[:, 256:512])
                # Wc = max_l S from the f16 copy (2x DVE mode)
                nc.vector.reduce_max(
                    wpe[:, 9 * s : 9 * s + 1], s_sb[:], axis=AxX
                )
                ps_st = pf.tile([128, 4 * 128], F32, tag="st", bufs=1)
                for j in range(4):
                    nc.tensor.matmul(
                        ps_st[:, j * 128 : (j + 1) * 128],
                        s_sb[:, j * 128 : (j + 1) * 128],
                        ident,
                        start=True,
                        stop=True,
                    )
                filler(1)
                nc.vector.reduce_max(
                    wpe[:, 9 * s + 1 : 9 * s + 9],
                    ps_st[:].rearrange("p (j g k) -> p j g k", j=4, k=NPAD),
                    axis=AxX,
                )
                nc.scalar.activation(
                    wce[:, s : s + 1], wpe[:, 9 * s : 9 * s + 1], AF.Exp
                )
                nc.scalar.activation(
                    ewx[:, 9 * s + 1 : 9 * s + 9],
                    wpe[:, 9 * s + 1 : 9 * s + 9],
                    AF.Exp,
                )
                nc.gpsimd.tensor_scalar_mul(
                    wcseg[:, 2 * s : 2 * s + 2],
                    in0=consts[:, C_IND + 2 * s : C_IND + 2 * s + 2],
                    scalar1=wce[:, s : s + 1],
                )

            # ---- denominators: Sc and t --------------------------------
            ps_sc = ps.tile([1, MPC], F32, tag="sp")
            nc.tensor.matmul(ps_sc[:], ones_col, wcseg[:], start=True, stop=True)

            tpart = work.tile([128, MPC], F16)
            with nc.allow_low_precision(reason="sum of 4 fp16 values, 5e-4 rel"):
                nc.vector.reduce_sum(
                    tpart[:].rearrange("p (s sl) -> p s sl", sl=2),
                    ewx[:]
                    .rearrange("p (s x) -> p s x", x=9)[:, :, 1:9]
                    .rearrange("p s (j sl) -> p s sl j", sl=2),
                    axis=AxX,
                )
            ps_t = ps.tile([1, MPC], F32, tag="sp")
            nc.tensor.matmul(ps_t[:], ones_col, tpart[:], start=True, stop=True)

            sct = work.tile([1, 2 * MPC], F16)
            nc.vector.tensor_copy(sct[:, :MPC], ps_sc[:])
            nc.vector.tensor_copy(sct[:, MPC:], ps_t[:])
            ps_bc = ps.tile([128, 2 * MPC], F32, tag="sp")
            nc.tensor.matmul(ps_bc[:], row1[:], sct[:], start=True, stop=True)
            inv = work.tile([128, 2 * MPC], F32)
            nc.vector.reciprocal(inv[:], ps_bc[:])

            # ---- pools -------------------------------------------------
            ps_ap = ps.tile([128, MPC], F32, tag="sp")
            for s in range(NSTACK):
                nc.tensor.matmul(
                    ps_ap[:, 2 * s : 2 * s + 2],
                    atomN[:, s, :],
                    wcseg[:, 2 * s : 2 * s + 2],
                    start=True,
                    stop=True,
                )
            # row-form residue pools packed 4 per PE column-group:
            # molecule g*4+sl accumulates in row 32*sl of psum tile g.
            prows = []
            for g in range(2):
                ps_pr = ps.tile([128, 128], F32, tag="sp")
                prows.append(ps_pr)
                for j in range(4):
                    for sl in range(4):
                        m = 4 * g + sl
                        ewc = 9 * (m // 2) + 1 + 2 * j + (m % 2)
                        nc.tensor.matmul(
                            ps_pr[32 * sl : 32 * sl + 1, :],
                            ewx[:, ewc : ewc + 1],
                            pnat[:, m * L + j * 128 : m * L + (j + 1) * 128],
                            start=(j == 0),
                            stop=(j == 3),
                            tile_position=(0, 32 * sl),
                        )
            ps_ppT = []
            for g in range(2):
                pr_sb = work.tile([128, 128], F16, tag=f"prsb{g}")
                nc.scalar.copy(pr_sb[:], prows[g][:])
                ps_pt = pf.tile([128, 128], F32, tag="st", bufs=1)
                nc.tensor.matmul(
                    ps_pt[:], pr_sb[:], ident, start=True, stop=True
                )
                ps_ppT.append(ps_pt)

            htop = work.tile([128, MPC], F16)
            nc.vector.tensor_mul(htop[:], ps_ap[:], inv[:, :MPC])
            hbot = work.tile([128, MPC], F16)
            for g in range(2):
                nc.vector.tensor_mul(
                    hbot[:, 4 * g : 4 * g + 4],
                    ps_ppT[g][:].rearrange("p (a b) -> p b a", b=32)[:, 0, :],
                    inv[:, MPC + 4 * g : MPC + 4 * g + 4],
                )

            # ---- MLP: single group of 8 molecules ----------------------
            h1 = work.tile([128, 4 * MPC], F16)
            h2 = work.tile([128, 2 * MPC], F16)
            for mc in range(4):
                ps_h1 = ps.tile([128, MPC], F32, tag="sp")
                nc.tensor.matmul(
                    ps_h1[:],
                    w1[:, mc * 128 : (mc + 1) * 128],
                    htop[:],
                    start=True,
                    stop=False,
                )
                nc.tensor.matmul(
                    ps_h1[:],
                    w1[:, H1 + mc * 128 : H1 + (mc + 1) * 128],
                    hbot[:],
                    start=False,
                    stop=True,
                )
                nc.scalar.activation(
                    h1[:, mc * MPC : (mc + 1) * MPC],
                    ps_h1[:],
                    AF.Relu,
                    bias=biasc[:, mc : mc + 1],
                )
            for mc2 in range(2):
                ps_h2 = ps.tile([128, MPC], F32, tag="sp")
                for kc in range(4):
                    nc.tensor.matmul(
                        ps_h2[:],
                        w2[:, kc * H2 + mc2 * 128 : kc * H2 + (mc2 + 1) * 128],
                        h1[:, kc * MPC : (kc + 1) * MPC],
                        start=(kc == 0),
                        stop=(kc == 3),
                    )
                nc.scalar.activation(
                    h2[:, mc2 * MPC : (mc2 + 1) * MPC],
                    ps_h2[:],
                    AF.Relu,
                    bias=biasc[:, 4 + mc2 : 4 + mc2 + 1],
                )
            ps_o = ps.tile([MPC, 1], F32, tag="sp")
            nc.tensor.matmul(
                ps_o[:], h2[:, :MPC], consts[:, C_WO : C_WO + 1], start=True, stop=False
            )
            nc.tensor.matmul(
                ps_o[:],
                h2[:, MPC : 2 * MPC],
                consts[:, C_WO + 1 : C_WO + 2],
                start=False,
                stop=True,
            )
            y_sb = work.tile([MPC, 1], F32)
            nc.scalar.add(y_sb[:], ps_o[:], biasc[0:MPC, 6:7])
            nc.sync.dma_start(d_y[:], y_sb[:])

    nc.compile()
    return nc


def _prep_inputs(atom_embed, protSeq_embed, atom_splits, W_att, W1, b1, W2, b2, Wo, bo):
    f16 = np.float16
    atom = np.asarray(atom_embed, dtype=np.float32)
    prot = np.asarray(protSeq_embed, dtype=np.float32)
    splits = np.asarray(atom_splits).astype(np.int64).ravel()
    order = np.argsort(splits, kind="stable")
    counts = np.bincount(splits, minlength=B)
    assert counts.max() <= NPAD, f"molecule with {counts.max()} atoms > NPAD={NPAD}"
    assert counts.min() >= 1, "empty molecule (reference produces NaN there)"
    offs = np.concatenate([[0], np.cumsum(counts)])

    atomP = np.empty((B, NPAD, D), np.float32)
    ind = np.zeros((B, NPAD), np.float32)
    for b in range(B):
        idx = order[offs[b] : offs[b + 1]]
        n = len(idx)
        atomP[b, :n] = atom[idx]
        atomP[b, n:] = atom[idx[0]]  # replicate a real atom: maxes stay exact
        ind[b, :n] = 1.0

    w_att = np.asarray(W_att, np.float32).astype(f16)  # [128, 128]
    w1h = (
        np.asarray(W1, np.float32)
        .reshape(2, 128, H1).transpose(1, 0, 2).reshape(128, 2 * H1).astype(f16)
    )
    w2h = (
        np.asarray(W2, np.float32)
        .reshape(4, 128, H2).transpose(1, 0, 2).reshape(128, 4 * H2).astype(f16)
    )
    w12h = np.ascontiguousarray(np.concatenate([w1h, w2h], axis=1))
    biasc = np.zeros((128, 8), np.float32)
    biasc[:, 0:4] = np.asarray(b1, np.float32).reshape(4, 128).T
    biasc[:, 4:6] = np.asarray(b2, np.float32).reshape(2, 128).T
    biasc[:, 6] = np.asarray(bo, np.float32).ravel()[0]
    woc = np.asarray(Wo, np.float32).reshape(2, 128).T.astype(f16)

    in_maps = []
    for c in range(NCORES):
        sl = slice(c * MPC, (c + 1) * MPC)
        protT_c = np.ascontiguousarray(
            prot[sl].transpose(0, 2, 1).astype(f16)
        )  # [MPC, 128, L]
        atomT_c = np.ascontiguousarray(atomP[sl].reshape(MPC * NPAD, D).T.astype(f16))
        atomN_c = np.ascontiguousarray(
            atomP[sl].reshape(NSTACK, 128, D).transpose(1, 0, 2)
            .reshape(128, NSTACK * D).astype(f16)
        )
        ind_c = np.zeros((128, MPC), f16)
        for m in range(MPC):
            s, slot = divmod(m, 2)
            ind_c[slot * NPAD : (slot + 1) * NPAD, m] = ind[c * MPC + m]
        consts = np.zeros((128, C_W), f16)
        consts[:, C_IDENT : C_IDENT + 128] = np.eye(128, dtype=f16)
        consts[:, C_IND : C_IND + MPC] = ind_c
        consts[:, C_ONES] = 1.0
        consts[:, C_WO : C_WO + 2] = woc
        im = {
            "early": np.ascontiguousarray(
                np.concatenate([atomT_c, w_att, consts], axis=1)
            ),
            "w12": w12h,
            "atomn": atomN_c,
            "biasc": biasc,
        }
        for s in range(NSTACK):
            im[f"protp{s}"] = np.ascontiguousarray(
                protT_c[2 * s : 2 * s + 2].transpose(1, 0, 2).reshape(128, 2 * L)
            )
        in_maps.append(im)
    return in_maps


def kernel(atom_embed, protSeq_embed, atom_splits, W_att, W1, b1, W2, b2, Wo, bo,
           _trace=False):
    if "nc" not in _PROGRAM_CACHE:
        _PROGRAM_CACHE["nc"] = _build_program()
    nc = _PROGRAM_CACHE["nc"]
    in_maps = _prep_inputs(
        atom_embed, protSeq_embed, atom_splits, W_att, W1, b1, W2, b2, Wo, bo
    )
    res = run_bass_kernel_spmd(
        nc, in_maps, core_ids=list(range(NCORES)), trace=_trace
    )
    _PROGRAM_CACHE["last_result"] = res
    out = np.concatenate([res.results[c]["y"] for c in range(NCORES)], axis=0)
    return out.astype(np.float32)
